# revision 1
# baseline (speedup 1.0000x reference)
"""Trainium2 Bass kernel for GNN message passing (SSIM-weighted edge aggregation).

Problem (per batch element, one NeuronCore each; B=8 across 8 cores):
  x, xp: [C=96, N=3136];  edge_index: idx_i/idx_j [N, K=18] node ids.
  For each (n, k): gather channel columns x_i = x[:, idx_i], x_j = x[:, idx_j],
  compute SSIM-like scalar sff(n,k) from channel stats, output
    Ex[c, n] = sum_k |xp_i - xp_j| * sff + sum_k xp_i + sum_k xp_j.

Device strategy:
  1. Build DRAM table [N, 256] f32 rows = [x.T(96) | xp.T(96) | mu | var | pad].
     (PE transpose + bn_stats for per-node channel mean/var.)
  2. Per chunk of 128 nodes: one SWDGE dma_gather of 4608 rows (i-side then
     j-side pairs; pair g lands on partition g%128, block g//128).
  3. DVE/ACT: pair products, xp diffs, per-pair sff scalars.
  4. K-reduction via TensorE: block one-hot matrices M (pair-partition -> node)
     as matmul weights; sff folded into M for the |dxp| term. PSUM accumulates
     all 54 matmuls -> [128 nodes, 96 ch] output tile, DMA to DRAM [N, C].
Host reassembles: out.T per core, stack -> [8, 96, 3136, 1].
"""

import os
import sys

import numpy as np

sys.path.insert(0, "/opt/trn_rl_repo")

B, C, N, K = 8, 96, 3136, 18
C1 = 1e-6
C2 = 1e-6
ROW = 256  # table row f32 elements (1024B; dma_gather needs %256B==0)
NCH = (N + 127) // 128  # 25 chunks (24 x 128 nodes + 1 x 64)
MAXCOLS = 2 * 128 * K // 16  # idx cols per chunk (288)

_nc_cache = None


def _build_nc():
    import concourse.bacc as bacc
    import concourse.mybir as mybir
    import concourse.tile as tile
    from concourse.library_config import mlp
    from concourse.masks import make_identity
    from contextlib import ExitStack

    f32 = mybir.dt.float32
    i16 = mybir.dt.int16
    AF = mybir.ActivationFunctionType
    OP = mybir.AluOpType
    AX = mybir.AxisListType

    nc = bacc.Bacc(None, target_bir_lowering=False, debug=False)

    x_d = nc.dram_tensor("x", [C, N], f32, kind="ExternalInput")
    xp_d = nc.dram_tensor("xp", [C, N], f32, kind="ExternalInput")
    idx_d = nc.dram_tensor("idx", [128, NCH, MAXCOLS], i16, kind="ExternalInput")
    mb_d = nc.dram_tensor("mbase", [128, 9, 64], f32, kind="ExternalInput")
    out_d = nc.dram_tensor("out", [N, C], f32, kind="ExternalOutput")

    with ExitStack() as ctx:
        tc = ctx.enter_context(tile.TileContext(nc))
        const = ctx.enter_context(tc.tile_pool(name="const", bufs=1))
        dpool = ctx.enter_context(tc.tile_pool(name="dram", bufs=1, space="DRAM"))
        build = ctx.enter_context(tc.tile_pool(name="build", bufs=3))
        gath = ctx.enter_context(tc.tile_pool(name="gath", bufs=2))
        work = ctx.enter_context(tc.tile_pool(name="work", bufs=2))
        stat = ctx.enter_context(tc.tile_pool(name="stat", bufs=2))
        mpool = ctx.enter_context(tc.tile_pool(name="mpool", bufs=4))
        outp = ctx.enter_context(tc.tile_pool(name="outp", bufs=3))
        psum = ctx.enter_context(tc.tile_pool(name="psum", bufs=2, space="PSUM"))
        psb = ctx.enter_context(tc.tile_pool(name="psb", bufs=2, space="PSUM"))

        nc.gpsimd.load_library(mlp)

        idx_sb = const.tile([128, NCH, MAXCOLS], i16)
        nc.sync.dma_start(out=idx_sb[:], in_=idx_d[:])
        mb_sb = const.tile([128, 9, 64], f32)
        nc.sync.dma_start(out=mb_sb[:], in_=mb_d[:])
        ident = const.tile([128, 128], f32)
        make_identity(nc, ident[:])

        table = dpool.tile([N, ROW], f32)

        # ---- phase 1: build the gather table --------------------------------
        for r in range(NCH):
            n0 = r * 128
            nr = min(128, N - n0)
            sl = slice(n0, n0 + nr)
            xc = build.tile([C, 128], f32, tag="xc")
            nc.sync.dma_start(out=xc[:, :nr], in_=x_d[:, sl])
            xpc = build.tile([C, 128], f32, tag="xpc")
            nc.sync.dma_start(out=xpc[:, :nr], in_=xp_d[:, sl])
            stage = build.tile([128, ROW], f32, tag="stage")
            tp = psb.tile([128, C], f32, tag="tp")
            nc.tensor.transpose(out=tp[:nr, :], in_=xc[:, :nr], identity=ident[:C, :C])
            nc.vector.tensor_copy(out=stage[:nr, 0:96], in_=tp[:nr, :])
            tp2 = psb.tile([128, C], f32, tag="tp2")
            nc.tensor.transpose(
                out=tp2[:nr, :], in_=xpc[:, :nr], identity=ident[:C, :C]
            )
            nc.vector.tensor_copy(out=stage[:nr, 96:192], in_=tp2[:nr, :])
            bst = stat.tile([128, 6], f32, tag="bst")
            nc.vector.bn_stats(out=bst[:nr, :], in_=stage[:nr, 0:96])
            mv = stat.tile([128, 2], f32, tag="mv")
            nc.vector.bn_aggr(out=mv[:nr, :], in_=bst[:nr, :])
            nc.vector.tensor_copy(out=stage[:nr, 192:194], in_=mv[:nr, :])
            nc.vector.memset(stage[:, 194:ROW], 0.0)
            nc.sync.dma_start(out=table[sl, :], in_=stage[:nr, :])

        # ---- phase 2: gather + per-edge math + K-reduction ------------------
        for c in range(NCH):
            n0 = c * 128
            nr = min(128, N - n0)
            L = nr * K  # pairs per side
            nb = L // 128  # blocks per side (18 or 9)
            ni = 2 * L  # gathered rows

            gt = gath.tile([128, 2 * nb, ROW], f32, tag="gt")
            nc.gpsimd.dma_gather(
                gt[:], table[:], idx_sb[:, c, 0 : ni // 16], ni, ni, ROW,
                single_packet=False,
            )

            x_i = gt[:, 0:nb, 0:96]
            x_j = gt[:, nb : 2 * nb, 0:96]
            xp_i = gt[:, 0:nb, 96:192]
            xp_j = gt[:, nb : 2 * nb, 96:192]
            mu_i = gt[:, 0:nb, 192]
            mu_j = gt[:, nb : 2 * nb, 192]
            var_i = gt[:, 0:nb, 193]
            var_j = gt[:, nb : 2 * nb, 193]

            P = work.tile([128, K, 96], f32, tag="P")
            nc.vector.tensor_mul(out=P[:, :nb, :], in0=x_i, in1=x_j)
            ps = stat.tile([128, K], f32, tag="ps")
            nc.vector.tensor_reduce(
                out=ps[:, :nb], in_=P[:, :nb, :], axis=AX.X, op=OP.add
            )
            D = work.tile([128, K, 96], f32, tag="D")
            nc.vector.tensor_sub(out=D[:, :nb, :], in0=xp_i, in1=xp_j)
            A = work.tile([128, K, 96], f32, tag="A")
            nc.scalar.activation(out=A[:, :nb, :], in_=D[:, :nb, :], func=AF.Abs)

            def st(tag):
                return stat.tile([128, K], f32, tag=tag, name=f"{tag}_{c}")

            mmt, t1, t2 = st("mmt"), st("t1"), st("t2")
            nc.vector.tensor_mul(out=mmt[:, :nb], in0=mu_i, in1=mu_j)
            nc.vector.tensor_mul(out=t1[:, :nb], in0=mu_i, in1=mu_i)
            nc.vector.tensor_mul(out=t2[:, :nb], in0=mu_j, in1=mu_j)
            den1, num1, r1, S1 = st("den1"), st("num1"), st("r1"), st("S1")
            nc.vector.scalar_tensor_tensor(
                out=den1[:, :nb], in0=t1[:, :nb], scalar=C1, in1=t2[:, :nb],
                op0=OP.add, op1=OP.add,
            )
            nc.vector.tensor_scalar(
                out=num1[:, :nb], in0=mmt[:, :nb], scalar1=2.0, scalar2=C1,
                op0=OP.mult, op1=OP.add,
            )
            nc.vector.reciprocal(out=r1[:, :nb], in_=den1[:, :nb])
            nc.vector.tensor_mul(out=S1[:, :nb], in0=num1[:, :nb], in1=r1[:, :nb])
            cov, num2, den2 = st("cov"), st("num2"), st("den2")
            nc.vector.scalar_tensor_tensor(
                out=cov[:, :nb], in0=ps[:, :nb], scalar=1.0 / 96.0, in1=mmt[:, :nb],
                op0=OP.mult, op1=OP.subtract,
            )
            nc.vector.tensor_scalar(
                out=num2[:, :nb], in0=cov[:, :nb], scalar1=2.0, scalar2=C2,
                op0=OP.mult, op1=OP.add,
            )
            nc.vector.scalar_tensor_tensor(
                out=den2[:, :nb], in0=var_i, scalar=C2, in1=var_j,
                op0=OP.add, op1=OP.add,
            )
            r2, S2, s12, sff = st("r2"), st("S2"), st("s12"), st("sff")
            nc.vector.reciprocal(out=r2[:, :nb], in_=den2[:, :nb])
            nc.vector.tensor_mul(out=S2[:, :nb], in0=num2[:, :nb], in1=r2[:, :nb])
            nc.vector.tensor_mul(out=s12[:, :nb], in0=S1[:, :nb], in1=S2[:, :nb])
            nc.vector.tensor_scalar(
                out=sff[:, :nb], in0=s12[:, :nb], scalar1=-1.0, scalar2=1.0,
                op0=OP.mult, op1=OP.add,
            )

            po = psum.tile([128, 96], f32, tag="po")
            for b in range(nb):
                mp = mpool.tile([128, 64], f32, tag="mp")
                nc.vector.tensor_scalar_mul(
                    out=mp[:], in0=mb_sb[:, b % 9, :], scalar1=sff[:, b : b + 1]
                )
                lo = 0 if b < 9 else 64
                nc.tensor.matmul(
                    out=po[lo : lo + 64, :], lhsT=mp[:], rhs=A[:, b, :],
                    start=(b % 9 == 0), stop=False,
                )
                nc.tensor.matmul(
                    out=po[lo : lo + 64, :], lhsT=mb_sb[:, b % 9, :],
                    rhs=gt[:, b, 96:192], start=False, stop=False,
                )
                nc.tensor.matmul(
                    out=po[lo : lo + 64, :], lhsT=mb_sb[:, b % 9, :],
                    rhs=gt[:, nb + b, 96:192], start=False, stop=(b % 9 == 8),
                )

            ot = outp.tile([128, 96], f32, tag="ot")
            nc.scalar.activation(out=ot[:nr, :], in_=po[:nr, :], func=AF.Copy)
            nc.sync.dma_start(out=out_d[n0 : n0 + nr, :], in_=ot[:nr, :])

    nc.compile()
    return nc


def _get_nc():
    global _nc_cache
    if _nc_cache is None:
        _nc_cache = _build_nc()
    return _nc_cache


def _build_idx(idx_i, idx_j):
    """idx_i/idx_j: [N, K] int -> [128, NCH, MAXCOLS] int16 wrapped layout."""
    chunks = []
    for c in range(NCH):
        n0 = c * 128
        n1 = min(n0 + 128, N)
        comb = np.concatenate(
            [idx_i[n0:n1].reshape(-1), idx_j[n0:n1].reshape(-1)]
        ).astype(np.int16)
        w = comb.reshape(-1, 16).T  # [16, ncols]; index g at (g%16, g//16)
        full = np.tile(w, (8, 1))  # replicate across the 8 q7 cores
        if full.shape[1] < MAXCOLS:
            full = np.pad(full, ((0, 0), (0, MAXCOLS - full.shape[1])))
        chunks.append(full)
    return np.ascontiguousarray(np.stack(chunks, axis=1))


def _mbase():
    p = np.arange(128)[:, None, None]
    bb = np.arange(9)[None, :, None]
    m = np.arange(64)[None, None, :]
    return np.ascontiguousarray(((bb * 128 + p) // K == m).astype(np.float32))


def kernel(x, x_p, edge_index):
    from concourse.bass_utils import run_bass_kernel_spmd

    xs = np.ascontiguousarray(x[..., 0], dtype=np.float32)  # [B, C, N]
    xps = np.ascontiguousarray(x_p[..., 0], dtype=np.float32)
    idx_j_all = np.asarray(edge_index[0])  # neighbors
    idx_i_all = np.asarray(edge_index[1])  # centers
    mb = _mbase()

    in_maps = []
    for b in range(B):
        in_maps.append(
            {
                "x": xs[b],
                "xp": xps[b],
                "idx": _build_idx(idx_i_all[b], idx_j_all[b]),
                "mbase": mb,
            }
        )

    nc = _get_nc()
    res = run_bass_kernel_spmd(nc, in_maps, list(range(B))).results
    out = np.stack([r["out"].T for r in res])  # [B, C, N]
    return np.ascontiguousarray(out[..., None]).astype(np.float32)


if __name__ == "__main__":
    # quick smoke test with random data
    rng = np.random.default_rng(0)
    x = rng.standard_normal((B, C, N, 1), dtype=np.float32)
    x_p = rng.random((B, C, N, 1), dtype=np.float32)
    ei = rng.integers(0, N, size=(2, B, N, K)).astype(np.int32)
    out = kernel(x, x_p, ei)
    print(out.shape, out.dtype)



# revision 2
# speedup vs baseline: 3.0983x; 3.0983x over previous
"""Trainium2 Bass kernel for GNN message passing (SSIM-weighted edge aggregation).

Problem (per batch element, one NeuronCore each; B=8 across 8 cores):
  x, xp: [C=96, N=3136];  edge_index: idx_i/idx_j [N, K=18] node ids.
  For each (n, k): gather channel columns x_i = x[:, idx_i], x_j = x[:, idx_j],
  compute SSIM-like scalar sff(n,k) from channel stats, output
    Ex[c, n] = sum_k |xp_i - xp_j| * sff + sum_k xp_i + sum_k xp_j.

Device strategy:
  1. Build DRAM table [N, 256] bf16 rows = [x.T(96) | xp.T(96) | mu | var | pad]
     (512B rows: full-rate DMA descriptors at half the f32 traffic).
  2. Per chunk of 128 nodes: gather 2*128*K rows via FOUR dma_gather calls on
     SWDGE queues 0-3 (each queue runs on its own Q7 core pair, so descriptor
     generation is ~4x parallel).  Pair g lands on partition g%128.
  3. DVE/ACT: per-pair products + stats chain -> sff [128, nb]; fold sff into
     the matmul rhs: R = |xp_i-xp_j| * sff + xp_i + xp_j (bf16).
  4. K-reduction via TensorE: per block one bf16 matmul with a constant
     one-hot [128 pairs -> 128 nodes] weight, PSUM-accumulated over blocks
     -> [128 nodes, 96 ch], DMA to DRAM [N, C].
Host reassembles: out.T per core, stack -> [8, 96, 3136, 1].
"""

import sys

import numpy as np

sys.path.insert(0, "/opt/trn_rl_repo")

B, C, N, K = 8, 96, 3136, 18
C1 = 1e-6
C2 = 1e-6
ROW = 256  # table row bf16 elements (512B; dma_gather needs %256B==0)
NCH = (N + 127) // 128  # 25 chunks (24 x 128 nodes + 1 x 64)
MAXCOLS = 2 * 128 * K // 16  # idx cols per chunk (288)

_nc_cache = None


def _qsizes(nblocks):
    """Split nblocks gather blocks across the 4 SWDGE queues."""
    base, rem = divmod(nblocks, 4)
    return [base + (1 if q < rem else 0) for q in range(4)]


def _build_nc():
    import concourse.bacc as bacc
    import concourse.mybir as mybir
    import concourse.tile as tile
    from concourse.library_config import mlp
    from concourse.masks import make_identity
    from contextlib import ExitStack

    f32 = mybir.dt.float32
    bf16 = mybir.dt.bfloat16
    i16 = mybir.dt.int16
    AF = mybir.ActivationFunctionType
    OP = mybir.AluOpType
    AX = mybir.AxisListType

    nc = bacc.Bacc(None, target_bir_lowering=False, debug=False, num_swdge_queues=4)

    x_d = nc.dram_tensor("x", [C, N], f32, kind="ExternalInput")
    xp_d = nc.dram_tensor("xp", [C, N], f32, kind="ExternalInput")
    idx_d = nc.dram_tensor("idx", [128, NCH, MAXCOLS], i16, kind="ExternalInput")
    mb_d = nc.dram_tensor("mbase", [128, K, 128], bf16, kind="ExternalInput")
    out_d = nc.dram_tensor("out", [N, C], f32, kind="ExternalOutput")

    with ExitStack() as ctx:
        tc = ctx.enter_context(tile.TileContext(nc))
        const = ctx.enter_context(tc.tile_pool(name="const", bufs=1))
        dpool = ctx.enter_context(tc.tile_pool(name="dram", bufs=1, space="DRAM"))
        build = ctx.enter_context(tc.tile_pool(name="build", bufs=3))
        gath = ctx.enter_context(tc.tile_pool(name="gath", bufs=2))
        work = ctx.enter_context(tc.tile_pool(name="work", bufs=2))
        stat = ctx.enter_context(tc.tile_pool(name="stat", bufs=2))
        outp = ctx.enter_context(tc.tile_pool(name="outp", bufs=3))
        psum = ctx.enter_context(tc.tile_pool(name="psum", bufs=2, space="PSUM"))
        psb = ctx.enter_context(tc.tile_pool(name="psb", bufs=2, space="PSUM"))

        nc.gpsimd.load_library(mlp)

        idx_sb = const.tile([128, NCH, MAXCOLS], i16)
        nc.sync.dma_start(out=idx_sb[:], in_=idx_d[:])
        mb_sb = const.tile([128, K, 128], bf16)
        nc.sync.dma_start(out=mb_sb[:], in_=mb_d[:])
        ident = const.tile([128, 128], f32)
        make_identity(nc, ident[:])

        table = dpool.tile([N, ROW], bf16)

        # ---- phase 1: build the gather table --------------------------------
        for r in range(NCH):
            n0 = r * 128
            nr = min(128, N - n0)
            sl = slice(n0, n0 + nr)
            xc = build.tile([C, 128], f32, tag="xc")
            nc.sync.dma_start(out=xc[:, :nr], in_=x_d[:, sl])
            xpc = build.tile([C, 128], f32, tag="xpc")
            nc.sync.dma_start(out=xpc[:, :nr], in_=xp_d[:, sl])
            stage = build.tile([128, ROW], bf16, tag="stage")
            tp = psb.tile([128, C], f32, tag="tp")
            nc.tensor.transpose(out=tp[:nr, :], in_=xc[:, :nr], identity=ident[:C, :C])
            nc.vector.tensor_copy(out=stage[:nr, 0:96], in_=tp[:nr, :])
            tp2 = psb.tile([128, C], f32, tag="tp2")
            nc.tensor.transpose(
                out=tp2[:nr, :], in_=xpc[:, :nr], identity=ident[:C, :C]
            )
            nc.scalar.activation(
                out=stage[:nr, 96:192], in_=tp2[:nr, :], func=AF.Copy
            )
            bst = stat.tile([128, 6], f32, tag="bst")
            nc.vector.bn_stats(out=bst[:nr, :], in_=tp[:nr, :])
            mv = stat.tile([128, 2], f32, tag="mv")
            nc.vector.bn_aggr(out=mv[:nr, :], in_=bst[:nr, :])
            nc.vector.tensor_copy(out=stage[:nr, 192:194], in_=mv[:nr, :])
            nc.sync.dma_start(out=table[sl, :], in_=stage[:nr, :])

        # ---- phase 2: gather + per-edge math + K-reduction ------------------
        for c in range(NCH):
            n0 = c * 128
            nr = min(128, N - n0)
            L = nr * K  # pairs per side
            nbs = L // 128  # blocks per side (18 or 9)

            gt = gath.tile([128, 2 * K, ROW], bf16, tag="gt")
            off_blk = 0
            off_col = 0
            for q, sz in enumerate(_qsizes(2 * nbs)):
                nio = sz * 128
                nc.gpsimd.dma_gather(
                    gt[:, off_blk : off_blk + sz, :],
                    table[:],
                    idx_sb[:, c, off_col : off_col + nio // 16],
                    nio,
                    nio,
                    ROW,
                    single_packet=False,
                    queue_num=q,
                )
                off_blk += sz
                off_col += nio // 16

            x_i = gt[:, 0:nbs, 0:96]
            x_j = gt[:, nbs : 2 * nbs, 0:96]
            xp_i = gt[:, 0:nbs, 96:192]
            xp_j = gt[:, nbs : 2 * nbs, 96:192]

            P = work.tile([128, K, 96], bf16, tag="P")
            nc.vector.tensor_mul(out=P[:, :nbs, :], in0=x_i, in1=x_j)
            ps = stat.tile([128, K], f32, tag="ps")
            nc.vector.tensor_reduce(
                out=ps[:, :nbs], in_=P[:, :nbs, :], axis=AX.X, op=OP.add
            )
            D = work.tile([128, K, 96], bf16, tag="D")
            nc.vector.tensor_sub(out=D[:, :nbs, :], in0=xp_i, in1=xp_j)
            A = work.tile([128, K, 96], bf16, tag="A")
            nc.scalar.activation(out=A[:, :nbs, :], in_=D[:, :nbs, :], func=AF.Abs)
            U = work.tile([128, K, 96], bf16, tag="U")
            nc.vector.tensor_add(out=U[:, :nbs, :], in0=xp_i, in1=xp_j)

            # per-pair channel stats (compact f32 copies of the strided cols)
            mus = stat.tile([128, 2 * K], f32, tag="mus")
            nc.vector.tensor_copy(out=mus[:, : 2 * nbs], in_=gt[:, 0 : 2 * nbs, 192])
            vas = stat.tile([128, 2 * K], f32, tag="vas")
            nc.vector.tensor_copy(out=vas[:, : 2 * nbs], in_=gt[:, 0 : 2 * nbs, 193])
            mu_i = mus[:, 0:nbs]
            mu_j = mus[:, nbs : 2 * nbs]
            var_i = vas[:, 0:nbs]
            var_j = vas[:, nbs : 2 * nbs]

            def st(tag):
                return stat.tile([128, K], f32, tag=tag, name=f"{tag}_{c}")

            mm, t1, t2 = st("mm"), st("t1"), st("t2")
            nc.vector.tensor_mul(out=mm[:, :nbs], in0=mu_i, in1=mu_j)
            nc.scalar.activation(out=t1[:, :nbs], in_=mu_i, func=AF.Square)
            nc.scalar.activation(out=t2[:, :nbs], in_=mu_j, func=AF.Square)
            den1, num1, r1, S1 = st("den1"), st("num1"), st("r1"), st("S1")
            nc.vector.scalar_tensor_tensor(
                out=den1[:, :nbs], in0=t1[:, :nbs], scalar=C1, in1=t2[:, :nbs],
                op0=OP.add, op1=OP.add,
            )
            nc.vector.tensor_scalar(
                out=num1[:, :nbs], in0=mm[:, :nbs], scalar1=2.0, scalar2=C1,
                op0=OP.mult, op1=OP.add,
            )
            nc.vector.reciprocal(out=r1[:, :nbs], in_=den1[:, :nbs])
            nc.vector.tensor_mul(out=S1[:, :nbs], in0=num1[:, :nbs], in1=r1[:, :nbs])
            cv, num2, den2 = st("cv"), st("num2"), st("den2")
            nc.vector.scalar_tensor_tensor(
                out=cv[:, :nbs], in0=ps[:, :nbs], scalar=1.0 / 96.0, in1=mm[:, :nbs],
                op0=OP.mult, op1=OP.subtract,
            )
            nc.vector.tensor_scalar(
                out=num2[:, :nbs], in0=cv[:, :nbs], scalar1=2.0, scalar2=C2,
                op0=OP.mult, op1=OP.add,
            )
            nc.vector.scalar_tensor_tensor(
                out=den2[:, :nbs], in0=var_i, scalar=C2, in1=var_j,
                op0=OP.add, op1=OP.add,
            )
            r2, S2, s12, sff = st("r2"), st("S2"), st("s12"), st("sff")
            nc.vector.reciprocal(out=r2[:, :nbs], in_=den2[:, :nbs])
            nc.vector.tensor_mul(out=S2[:, :nbs], in0=num2[:, :nbs], in1=r2[:, :nbs])
            nc.vector.tensor_mul(out=s12[:, :nbs], in0=S1[:, :nbs], in1=S2[:, :nbs])
            nc.vector.tensor_scalar(
                out=sff[:, :nbs], in0=s12[:, :nbs], scalar1=-1.0, scalar2=1.0,
                op0=OP.mult, op1=OP.add,
            )

            # fold sff into the matmul rhs: R2 = A*sff + xp_i + xp_j
            R = work.tile([128, K, 96], bf16, tag="R")
            nc.vector.tensor_mul(
                out=R[:, :nbs, :],
                in0=A[:, :nbs, :],
                in1=sff[:, :nbs].to_broadcast((128, nbs, 96)),
            )
            R2 = work.tile([128, K, 96], bf16, tag="R2")
            nc.vector.tensor_add(out=R2[:, :nbs, :], in0=R[:, :nbs, :], in1=U[:, :nbs, :])

            po = psum.tile([128, 96], f32, tag="po")
            for b in range(nbs):
                nc.tensor.matmul(
                    out=po[:, :], lhsT=mb_sb[:, b, :], rhs=R2[:, b, :],
                    start=(b == 0), stop=(b == nbs - 1),
                )

            ot = outp.tile([128, 96], f32, tag="ot")
            nc.scalar.activation(out=ot[:nr, :], in_=po[:nr, :], func=AF.Copy)
            nc.sync.dma_start(out=out_d[n0 : n0 + nr, :], in_=ot[:nr, :])

    nc.compile()
    return nc


def _get_nc():
    global _nc_cache
    if _nc_cache is None:
        _nc_cache = _build_nc()
    return _nc_cache


def _build_idx(idx_i, idx_j):
    """idx_i/idx_j: [N, K] int -> [128, NCH, MAXCOLS] int16 wrapped layout.

    Per chunk the 2*nr*K indices (i-side then j-side) are split into 4
    contiguous block-ranges (one per SWDGE queue), each independently wrapped
    into 16 partitions and replicated across the 8 Q7 core pairs.
    """
    chunks = []
    for c in range(NCH):
        n0 = c * 128
        n1 = min(n0 + 128, N)
        comb = np.concatenate(
            [idx_i[n0:n1].reshape(-1), idx_j[n0:n1].reshape(-1)]
        ).astype(np.int16)
        nblocks = comb.size // 128
        cols = []
        off = 0
        for sz in _qsizes(nblocks):
            seg = comb[off * 128 : (off + sz) * 128]
            off += sz
            cols.append(seg.reshape(-1, 16).T)  # [16, sz*8]
        w = np.concatenate(cols, axis=1)
        full = np.tile(w, (8, 1))  # replicate across the 8 q7 cores
        if full.shape[1] < MAXCOLS:
            full = np.pad(full, ((0, 0), (0, MAXCOLS - full.shape[1])))
        chunks.append(full)
    return np.ascontiguousarray(np.stack(chunks, axis=1))


def _mbase():
    import ml_dtypes

    p = np.arange(128)[:, None, None]
    bb = np.arange(K)[None, :, None]
    m = np.arange(128)[None, None, :]
    mb = ((bb * 128 + p) // K == m).astype(np.float32)
    return np.ascontiguousarray(mb.astype(ml_dtypes.bfloat16))


def kernel(x, x_p, edge_index):
    from concourse.bass_utils import run_bass_kernel_spmd

    xs = np.ascontiguousarray(x[..., 0], dtype=np.float32)  # [B, C, N]
    xps = np.ascontiguousarray(x_p[..., 0], dtype=np.float32)
    idx_j_all = np.asarray(edge_index[0])  # neighbors
    idx_i_all = np.asarray(edge_index[1])  # centers
    mb = _mbase()

    in_maps = []
    for b in range(B):
        in_maps.append(
            {
                "x": xs[b],
                "xp": xps[b],
                "idx": _build_idx(idx_i_all[b], idx_j_all[b]),
                "mbase": mb,
            }
        )

    nc = _get_nc()
    res = run_bass_kernel_spmd(nc, in_maps, list(range(B))).results
    out = np.stack([r["out"].T for r in res])  # [B, C, N]
    return np.ascontiguousarray(out[..., None]).astype(np.float32)


if __name__ == "__main__":
    # quick smoke test with random data
    rng = np.random.default_rng(0)
    x = rng.standard_normal((B, C, N, 1), dtype=np.float32)
    x_p = rng.random((B, C, N, 1), dtype=np.float32)
    ei = rng.integers(0, N, size=(2, B, N, K)).astype(np.int32)
    out = kernel(x, x_p, ei)
    print(out.shape, out.dtype)


# revision 6
# speedup vs baseline: 3.1140x; 1.0050x over previous
"""Trainium2 Bass kernel for GNN message passing (SSIM-weighted edge aggregation).

Problem (per batch element, one NeuronCore each; B=8 across 8 cores):
  x, xp: [C=96, N=3136];  edge_index: idx_i/idx_j [N, K=18] node ids.
  For each (n, k): gather channel columns x_i = x[:, idx_i], x_j = x[:, idx_j],
  compute SSIM-like scalar sff(n,k) from channel stats, output
    Ex[c, n] = sum_k |xp_i - xp_j| * sff + sum_k xp_i + sum_k xp_j.

Device strategy:
  1. Build DRAM table [N, 256] bf16 rows = [x.T(96) | xp.T(96) | mu | var | pad]
     (512B rows: full-rate DMA descriptors at half the f32 traffic).
  2. Per chunk of 128 nodes: gather 2*128*K rows via FOUR dma_gather calls on
     SWDGE queues 0-3 (each queue runs on its own Q7 core pair, so descriptor
     generation is ~4x parallel).  Pair g lands on partition g%128.
  3. DVE/ACT: per-pair products + stats chain -> sff [128, nb]; fold sff into
     the matmul rhs: R = |xp_i-xp_j| * sff + xp_i + xp_j (bf16).
  4. K-reduction via TensorE: per block one bf16 matmul with a constant
     one-hot [128 pairs -> 128 nodes] weight, PSUM-accumulated over blocks
     -> [128 nodes, 96 ch], DMA to DRAM [N, C].
Host reassembles: out.T per core, stack -> [8, 96, 3136, 1].
"""

import sys

import numpy as np

sys.path.insert(0, "/opt/trn_rl_repo")

B, C, N, K = 8, 96, 3136, 18
C1 = 1e-6
C2 = 1e-6
ROW = 256  # table row bf16 elements (512B; dma_gather needs %256B==0)
NCH = (N + 127) // 128  # 25 chunks (24 x 128 nodes + 1 x 64)
MAXCOLS = 2 * 128 * K // 16  # idx cols per chunk (288)

_nc_cache = None


def _qsizes(nblocks):
    """Split nblocks gather blocks across the 4 SWDGE queues."""
    base, rem = divmod(nblocks, 4)
    return [base + (1 if q < rem else 0) for q in range(4)]


def _build_nc():
    import concourse.bacc as bacc
    import concourse.mybir as mybir
    import concourse.tile as tile
    from concourse.library_config import mlp
    from concourse.masks import make_identity
    from contextlib import ExitStack

    f32 = mybir.dt.float32
    bf16 = mybir.dt.bfloat16
    i16 = mybir.dt.int16
    AF = mybir.ActivationFunctionType
    OP = mybir.AluOpType
    AX = mybir.AxisListType

    nc = bacc.Bacc(None, target_bir_lowering=False, debug=False, num_swdge_queues=4)

    x_d = nc.dram_tensor("x", [C, N], f32, kind="ExternalInput")
    xp_d = nc.dram_tensor("xp", [C, N], f32, kind="ExternalInput")
    idx_d = nc.dram_tensor("idx", [128, NCH, MAXCOLS], i16, kind="ExternalInput")
    mb_d = nc.dram_tensor("mbase", [128, K, 128], bf16, kind="ExternalInput")
    out_d = nc.dram_tensor("out", [N, C], f32, kind="ExternalOutput")

    with ExitStack() as ctx:
        tc = ctx.enter_context(tile.TileContext(nc))
        const = ctx.enter_context(tc.tile_pool(name="const", bufs=1))
        dpool = ctx.enter_context(tc.tile_pool(name="dram", bufs=1, space="DRAM"))
        build = ctx.enter_context(tc.tile_pool(name="build", bufs=3))
        gath = ctx.enter_context(tc.tile_pool(name="gath", bufs=3))
        work = ctx.enter_context(tc.tile_pool(name="work", bufs=2))
        stat = ctx.enter_context(tc.tile_pool(name="stat", bufs=2))
        outp = ctx.enter_context(tc.tile_pool(name="outp", bufs=3))
        psum = ctx.enter_context(tc.tile_pool(name="psum", bufs=2, space="PSUM"))
        psb = ctx.enter_context(tc.tile_pool(name="psb", bufs=2, space="PSUM"))

        nc.gpsimd.load_library(mlp)

        idx_sb = const.tile([128, NCH, MAXCOLS], i16)
        nc.sync.dma_start(out=idx_sb[:], in_=idx_d[:])
        mb_sb = const.tile([128, K, 128], bf16)
        nc.sync.dma_start(out=mb_sb[:], in_=mb_d[:])
        ident = const.tile([128, 128], f32)
        make_identity(nc, ident[:])

        table = dpool.tile([N, ROW], bf16)

        # ---- phase 1: build the gather table --------------------------------
        for r in range(NCH):
            n0 = r * 128
            nr = min(128, N - n0)
            sl = slice(n0, n0 + nr)
            xc = build.tile([C, 128], f32, tag="xc")
            nc.sync.dma_start(out=xc[:, :nr], in_=x_d[:, sl])
            xpc = build.tile([C, 128], f32, tag="xpc")
            nc.sync.dma_start(out=xpc[:, :nr], in_=xp_d[:, sl])
            stage = build.tile([128, ROW], bf16, tag="stage")
            tp = psb.tile([128, C], f32, tag="tp")
            nc.tensor.transpose(out=tp[:nr, :], in_=xc[:, :nr], identity=ident[:C, :C])
            nc.vector.tensor_copy(out=stage[:nr, 0:96], in_=tp[:nr, :])
            tp2 = psb.tile([128, C], f32, tag="tp2")
            nc.tensor.transpose(
                out=tp2[:nr, :], in_=xpc[:, :nr], identity=ident[:C, :C]
            )
            nc.scalar.activation(
                out=stage[:nr, 96:192], in_=tp2[:nr, :], func=AF.Copy
            )
            bst = stat.tile([128, 6], f32, tag="bst")
            nc.vector.bn_stats(out=bst[:nr, :], in_=tp[:nr, :])
            mv = stat.tile([128, 2], f32, tag="mv")
            nc.vector.bn_aggr(out=mv[:nr, :], in_=bst[:nr, :])
            nc.vector.tensor_copy(out=stage[:nr, 192:194], in_=mv[:nr, :])
            nc.sync.dma_start(out=table[sl, :], in_=stage[:nr, :])

        # ---- phase 2: gather + per-edge math + K-reduction ------------------
        for c in range(NCH):
            n0 = c * 128
            nr = min(128, N - n0)
            L = nr * K  # pairs per side
            nbs = L // 128  # blocks per side (18 or 9)

            gt = gath.tile([128, 2 * K, ROW], bf16, tag="gt")
            off_blk = 0
            off_col = 0
            for q, sz in enumerate(_qsizes(2 * nbs)):
                nio = sz * 128
                nc.gpsimd.dma_gather(
                    gt[:, off_blk : off_blk + sz, :],
                    table[:],
                    idx_sb[:, c, off_col : off_col + nio // 16],
                    nio,
                    nio,
                    ROW,
                    single_packet=False,
                    queue_num=q,
                )
                off_blk += sz
                off_col += nio // 16

            x_i = gt[:, 0:nbs, 0:96]
            x_j = gt[:, nbs : 2 * nbs, 0:96]
            xp_i = gt[:, 0:nbs, 96:192]
            xp_j = gt[:, nbs : 2 * nbs, 96:192]

            P = work.tile([128, K, 96], bf16, tag="P")
            nc.vector.tensor_mul(out=P[:, :nbs, :], in0=x_i, in1=x_j)
            ps = stat.tile([128, K], bf16, tag="ps")
            with nc.allow_low_precision(reason="cov tolerates bf16 channel sums"):
                nc.vector.tensor_reduce(
                    out=ps[:, :nbs], in_=P[:, :nbs, :], axis=AX.X, op=OP.add
                )
            D = work.tile([128, K, 96], bf16, tag="D")
            nc.vector.tensor_sub(out=D[:, :nbs, :], in0=xp_i, in1=xp_j)
            A = work.tile([128, K, 96], bf16, tag="A")
            nc.scalar.activation(out=A[:, :nbs, :], in_=D[:, :nbs, :], func=AF.Abs)

            # per-pair channel stats (compact f32 copies of the strided cols)
            mus = stat.tile([128, 2 * K], f32, tag="mus")
            nc.scalar.activation(
                out=mus[:, : 2 * nbs], in_=gt[:, 0 : 2 * nbs, 192], func=AF.Copy
            )
            vas = stat.tile([128, 2 * K], f32, tag="vas")
            nc.scalar.activation(
                out=vas[:, : 2 * nbs], in_=gt[:, 0 : 2 * nbs, 193], func=AF.Copy
            )
            mu_i = mus[:, 0:nbs]
            mu_j = mus[:, nbs : 2 * nbs]
            var_i = vas[:, 0:nbs]
            var_j = vas[:, nbs : 2 * nbs]

            def st(tag):
                return stat.tile([128, K], f32, tag=tag, name=f"{tag}_{c}")

            mm, t1, t2 = st("mm"), st("t1"), st("t2")
            nc.vector.tensor_mul(out=mm[:, :nbs], in0=mu_i, in1=mu_j)
            nc.scalar.activation(out=t1[:, :nbs], in_=mu_i, func=AF.Square)
            nc.scalar.activation(out=t2[:, :nbs], in_=mu_j, func=AF.Square)
            den1, num1, r1, S1 = st("den1"), st("num1"), st("r1"), st("S1")
            nc.vector.scalar_tensor_tensor(
                out=den1[:, :nbs], in0=t1[:, :nbs], scalar=C1, in1=t2[:, :nbs],
                op0=OP.add, op1=OP.add,
            )
            nc.vector.tensor_scalar(
                out=num1[:, :nbs], in0=mm[:, :nbs], scalar1=2.0, scalar2=C1,
                op0=OP.mult, op1=OP.add,
            )
            nc.vector.reciprocal(out=r1[:, :nbs], in_=den1[:, :nbs])
            nc.vector.tensor_mul(out=S1[:, :nbs], in0=num1[:, :nbs], in1=r1[:, :nbs])
            cv, num2, den2 = st("cv"), st("num2"), st("den2")
            nc.vector.scalar_tensor_tensor(
                out=cv[:, :nbs], in0=ps[:, :nbs], scalar=1.0 / 96.0, in1=mm[:, :nbs],
                op0=OP.mult, op1=OP.subtract,
            )
            nc.vector.tensor_scalar(
                out=num2[:, :nbs], in0=cv[:, :nbs], scalar1=2.0, scalar2=C2,
                op0=OP.mult, op1=OP.add,
            )
            nc.vector.scalar_tensor_tensor(
                out=den2[:, :nbs], in0=var_i, scalar=C2, in1=var_j,
                op0=OP.add, op1=OP.add,
            )
            r2, S2, s12, sff = st("r2"), st("S2"), st("s12"), st("sff")
            nc.vector.reciprocal(out=r2[:, :nbs], in_=den2[:, :nbs])
            nc.vector.tensor_mul(out=S2[:, :nbs], in0=num2[:, :nbs], in1=r2[:, :nbs])
            nc.vector.tensor_mul(out=s12[:, :nbs], in0=S1[:, :nbs], in1=S2[:, :nbs])
            nc.vector.tensor_scalar(
                out=sff[:, :nbs], in0=s12[:, :nbs], scalar1=-1.0, scalar2=1.0,
                op0=OP.mult, op1=OP.add,
            )

            # fold sff into the matmul rhs: R = A*sff; xp_i/xp_j summed by
            # extra matmuls sharing each block's one-hot weight load.
            R = work.tile([128, K, 96], bf16, tag="R")
            nc.vector.tensor_mul(
                out=R[:, :nbs, :],
                in0=A[:, :nbs, :],
                in1=sff[:, :nbs].to_broadcast((128, nbs, 96)),
            )

            po = psum.tile([128, 96], f32, tag="po")
            for b in range(nbs):
                nc.tensor.matmul(
                    out=po[:, :], lhsT=mb_sb[:, b, :], rhs=R[:, b, :],
                    start=(b == 0), stop=False,
                )
                nc.tensor.matmul(
                    out=po[:, :], lhsT=mb_sb[:, b, :], rhs=gt[:, b, 96:192],
                    start=False, stop=False,
                )
                nc.tensor.matmul(
                    out=po[:, :], lhsT=mb_sb[:, b, :], rhs=gt[:, nbs + b, 96:192],
                    start=False, stop=(b == nbs - 1),
                )

            ot = outp.tile([128, 96], f32, tag="ot")
            nc.scalar.activation(out=ot[:nr, :], in_=po[:nr, :], func=AF.Copy)
            nc.sync.dma_start(out=out_d[n0 : n0 + nr, :], in_=ot[:nr, :])

    nc.compile()
    return nc


def _get_nc():
    global _nc_cache
    if _nc_cache is None:
        _nc_cache = _build_nc()
    return _nc_cache


def _build_idx(idx_i, idx_j):
    """idx_i/idx_j: [N, K] int -> [128, NCH, MAXCOLS] int16 wrapped layout.

    Per chunk the 2*nr*K indices (i-side then j-side) are split into 4
    contiguous block-ranges (one per SWDGE queue), each independently wrapped
    into 16 partitions and replicated across the 8 Q7 core pairs.
    """
    chunks = []
    for c in range(NCH):
        n0 = c * 128
        n1 = min(n0 + 128, N)
        comb = np.concatenate(
            [idx_i[n0:n1].reshape(-1), idx_j[n0:n1].reshape(-1)]
        ).astype(np.int16)
        nblocks = comb.size // 128
        cols = []
        off = 0
        for sz in _qsizes(nblocks):
            seg = comb[off * 128 : (off + sz) * 128]
            off += sz
            cols.append(seg.reshape(-1, 16).T)  # [16, sz*8]
        w = np.concatenate(cols, axis=1)
        full = np.tile(w, (8, 1))  # replicate across the 8 q7 cores
        if full.shape[1] < MAXCOLS:
            full = np.pad(full, ((0, 0), (0, MAXCOLS - full.shape[1])))
        chunks.append(full)
    return np.ascontiguousarray(np.stack(chunks, axis=1))


def _mbase():
    import ml_dtypes

    p = np.arange(128)[:, None, None]
    bb = np.arange(K)[None, :, None]
    m = np.arange(128)[None, None, :]
    mb = ((bb * 128 + p) // K == m).astype(np.float32)
    return np.ascontiguousarray(mb.astype(ml_dtypes.bfloat16))


def kernel(x, x_p, edge_index):
    from concourse.bass_utils import run_bass_kernel_spmd

    xs = np.ascontiguousarray(x[..., 0], dtype=np.float32)  # [B, C, N]
    xps = np.ascontiguousarray(x_p[..., 0], dtype=np.float32)
    idx_j_all = np.asarray(edge_index[0])  # neighbors
    idx_i_all = np.asarray(edge_index[1])  # centers
    mb = _mbase()

    in_maps = []
    for b in range(B):
        in_maps.append(
            {
                "x": xs[b],
                "xp": xps[b],
                "idx": _build_idx(idx_i_all[b], idx_j_all[b]),
                "mbase": mb,
            }
        )

    nc = _get_nc()
    res = run_bass_kernel_spmd(nc, in_maps, list(range(B))).results
    out = np.stack([r["out"].T for r in res])  # [B, C, N]
    return np.ascontiguousarray(out[..., None]).astype(np.float32)


if __name__ == "__main__":
    # quick smoke test with random data
    rng = np.random.default_rng(0)
    x = rng.standard_normal((B, C, N, 1), dtype=np.float32)
    x_p = rng.random((B, C, N, 1), dtype=np.float32)
    ei = rng.integers(0, N, size=(2, B, N, K)).astype(np.int32)
    out = kernel(x, x_p, ei)
    print(out.shape, out.dtype)


# revision 9
# speedup vs baseline: 3.3900x; 1.0886x over previous
"""Trainium2 Bass kernel for GNN message passing (SSIM-weighted edge aggregation).

Problem (per batch element, one NeuronCore each; B=8 across 8 cores):
  x, xp: [C=96, N=3136];  edge_index: idx_i/idx_j [N, K=18] node ids.
  For each (n, k): gather channel columns x_i = x[:, idx_i], x_j = x[:, idx_j],
  compute SSIM-like scalar sff(n,k) from channel stats, output
    Ex[c, n] = sum_k |xp_i - xp_j| * sff + sum_k xp_i + sum_k xp_j.

Device strategy:
  1. Build DRAM table [N, 256] bf16 rows = [x.T(96) | xp.T(96) | mu | var | pad]
     (512B rows: full-rate DMA descriptors at half the f32 traffic).
  2. Per chunk of 128 nodes: gather 2*128*K rows via FOUR dma_gather calls on
     SWDGE queues 0-3 (each queue runs on its own Q7 core pair, so descriptor
     generation is ~4x parallel).  Pair g lands on partition g%128.
  3. DVE/ACT: per-pair products + stats chain -> sff [128, nb]; fold sff into
     the matmul rhs: R = |xp_i-xp_j| * sff + xp_i + xp_j (bf16).
  4. K-reduction via TensorE: per block one bf16 matmul with a constant
     one-hot [128 pairs -> 128 nodes] weight, PSUM-accumulated over blocks
     -> [128 nodes, 96 ch], DMA to DRAM [N, C].
Host reassembles: out.T per core, stack -> [8, 96, 3136, 1].
"""

import sys

import numpy as np

sys.path.insert(0, "/opt/trn_rl_repo")

B, C, N, K = 8, 96, 3136, 18
C1 = 1e-6
C2 = 1e-6
ROW = 256  # table row bf16 elements (512B; dma_gather needs %256B==0)
NCH = (N + 127) // 128  # 25 chunks (24 x 128 nodes + 1 x 64)
MAXCOLS = 2 * 128 * K // 16  # idx cols per chunk (288)

_nc_cache = None


def _qsizes(nblocks):
    """Split nblocks gather blocks across the 4 SWDGE queues."""
    base, rem = divmod(nblocks, 4)
    return [base + (1 if q < rem else 0) for q in range(4)]


def _build_nc():
    import concourse.bacc as bacc
    import concourse.mybir as mybir
    import concourse.tile as tile
    from concourse.library_config import mlp
    from concourse.masks import make_identity
    from contextlib import ExitStack

    f32 = mybir.dt.float32
    bf16 = mybir.dt.bfloat16
    i16 = mybir.dt.int16
    AF = mybir.ActivationFunctionType
    OP = mybir.AluOpType
    AX = mybir.AxisListType

    nc = bacc.Bacc(None, target_bir_lowering=False, debug=False, num_swdge_queues=4)

    x_d = nc.dram_tensor("x", [C, N], f32, kind="ExternalInput")
    xp_d = nc.dram_tensor("xp", [C, N], f32, kind="ExternalInput")
    idx_d = nc.dram_tensor("idx", [128, NCH, MAXCOLS], i16, kind="ExternalInput")
    mb_d = nc.dram_tensor("mbase", [128, K, 128], bf16, kind="ExternalInput")
    out_d = nc.dram_tensor("out", [N, C], f32, kind="ExternalOutput")

    with ExitStack() as ctx:
        tc = ctx.enter_context(tile.TileContext(nc))
        const = ctx.enter_context(tc.tile_pool(name="const", bufs=1))
        dpool = ctx.enter_context(tc.tile_pool(name="dram", bufs=1, space="DRAM"))
        build = ctx.enter_context(tc.tile_pool(name="build", bufs=3))
        gath = ctx.enter_context(tc.tile_pool(name="gath", bufs=3))
        work = ctx.enter_context(tc.tile_pool(name="work", bufs=2))
        stat = ctx.enter_context(tc.tile_pool(name="stat", bufs=2))
        outp = ctx.enter_context(tc.tile_pool(name="outp", bufs=3))
        psum = ctx.enter_context(tc.tile_pool(name="psum", bufs=3, space="PSUM"))
        psb = ctx.enter_context(tc.tile_pool(name="psb", bufs=2, space="PSUM"))

        nc.gpsimd.load_library(mlp)

        idx_sb = const.tile([128, NCH, MAXCOLS], i16)
        nc.sync.dma_start(out=idx_sb[:], in_=idx_d[:])
        mb_sb = const.tile([128, K, 128], bf16)
        nc.sync.dma_start(out=mb_sb[:], in_=mb_d[:])
        ident = const.tile([128, 128], f32)
        make_identity(nc, ident[:])

        table = dpool.tile([N, ROW], bf16)

        # ---- phase 1: build the gather table --------------------------------
        for r in range(NCH):
            n0 = r * 128
            nr = min(128, N - n0)
            sl = slice(n0, n0 + nr)
            xc = build.tile([C, 128], f32, tag="xc")
            nc.sync.dma_start(out=xc[:, :nr], in_=x_d[:, sl])
            xpc = build.tile([C, 128], f32, tag="xpc")
            nc.sync.dma_start(out=xpc[:, :nr], in_=xp_d[:, sl])
            stage = build.tile([128, ROW], bf16, tag="stage")
            tp = psb.tile([128, C], f32, tag="tp")
            nc.tensor.transpose(out=tp[:nr, :], in_=xc[:, :nr], identity=ident[:C, :C])
            nc.vector.tensor_copy(out=stage[:nr, 0:96], in_=tp[:nr, :])
            tp2 = psb.tile([128, C], f32, tag="tp2")
            nc.tensor.transpose(
                out=tp2[:nr, :], in_=xpc[:, :nr], identity=ident[:C, :C]
            )
            nc.scalar.activation(
                out=stage[:nr, 96:192], in_=tp2[:nr, :], func=AF.Copy
            )
            bst = stat.tile([128, 6], f32, tag="bst")
            nc.vector.bn_stats(out=bst[:nr, :], in_=tp[:nr, :])
            mv = stat.tile([128, 2], f32, tag="mv")
            nc.vector.bn_aggr(out=mv[:nr, :], in_=bst[:nr, :])
            nc.vector.tensor_copy(out=stage[:nr, 192:194], in_=mv[:nr, :])
            nc.sync.dma_start(out=table[sl, :], in_=stage[:nr, :])

        # ---- phase 2: gather + per-edge math + K-reduction ------------------
        for c in range(NCH):
            n0 = c * 128
            nr = min(128, N - n0)
            L = nr * K  # pairs per side
            nbs = L // 128  # blocks per side (18 or 9)

            gt = gath.tile([128, 2 * K, ROW], bf16, tag="gt")
            off_blk = 0
            off_col = 0
            for q, sz in enumerate(_qsizes(2 * nbs)):
                nio = sz * 128
                nc.gpsimd.dma_gather(
                    gt[:, off_blk : off_blk + sz, :],
                    table[:],
                    idx_sb[:, c, off_col : off_col + nio // 16],
                    nio,
                    nio,
                    ROW,
                    single_packet=False,
                    queue_num=q,
                )
                off_blk += sz
                off_col += nio // 16

            x_i = gt[:, 0:nbs, 0:96]
            x_j = gt[:, nbs : 2 * nbs, 0:96]
            xp_i = gt[:, 0:nbs, 96:192]
            xp_j = gt[:, nbs : 2 * nbs, 96:192]

            P = work.tile([128, K, 96], bf16, tag="P")
            nc.vector.tensor_mul(out=P[:, :nbs, :], in0=x_i, in1=x_j)
            ps = stat.tile([128, K], f32, tag="ps")
            nc.vector.tensor_reduce(
                out=ps[:, :nbs], in_=P[:, :nbs, :], axis=AX.X, op=OP.add
            )
            D = work.tile([128, K, 96], bf16, tag="D")
            nc.vector.tensor_sub(out=D[:, :nbs, :], in0=xp_i, in1=xp_j)
            A = work.tile([128, K, 96], bf16, tag="A")
            nc.scalar.activation(out=A[:, :nbs, :], in_=D[:, :nbs, :], func=AF.Abs)
            U = work.tile([128, K, 96], bf16, tag="U")
            nc.vector.tensor_add(out=U[:, :nbs, :], in0=xp_i, in1=xp_j)

            # per-pair channel stats (compact f32 copies of the strided cols)
            mus = stat.tile([128, 2 * K], f32, tag="mus")
            nc.vector.tensor_copy(out=mus[:, : 2 * nbs], in_=gt[:, 0 : 2 * nbs, 192])
            vas = stat.tile([128, 2 * K], f32, tag="vas")
            nc.vector.tensor_copy(out=vas[:, : 2 * nbs], in_=gt[:, 0 : 2 * nbs, 193])
            mu_i = mus[:, 0:nbs]
            mu_j = mus[:, nbs : 2 * nbs]
            var_i = vas[:, 0:nbs]
            var_j = vas[:, nbs : 2 * nbs]

            def st(tag):
                return stat.tile([128, K], f32, tag=tag, name=f"{tag}_{c}")

            mm, t1, t2 = st("mm"), st("t1"), st("t2")
            nc.vector.tensor_mul(out=mm[:, :nbs], in0=mu_i, in1=mu_j)
            nc.scalar.activation(out=t1[:, :nbs], in_=mu_i, func=AF.Square)
            nc.scalar.activation(out=t2[:, :nbs], in_=mu_j, func=AF.Square)
            den1, num1, r1, S1 = st("den1"), st("num1"), st("r1"), st("S1")
            nc.vector.scalar_tensor_tensor(
                out=den1[:, :nbs], in0=t1[:, :nbs], scalar=C1, in1=t2[:, :nbs],
                op0=OP.add, op1=OP.add,
            )
            nc.vector.tensor_scalar(
                out=num1[:, :nbs], in0=mm[:, :nbs], scalar1=2.0, scalar2=C1,
                op0=OP.mult, op1=OP.add,
            )
            nc.vector.reciprocal(out=r1[:, :nbs], in_=den1[:, :nbs])
            nc.vector.tensor_mul(out=S1[:, :nbs], in0=num1[:, :nbs], in1=r1[:, :nbs])
            cv, num2, den2 = st("cv"), st("num2"), st("den2")
            nc.vector.scalar_tensor_tensor(
                out=cv[:, :nbs], in0=ps[:, :nbs], scalar=1.0 / 96.0, in1=mm[:, :nbs],
                op0=OP.mult, op1=OP.subtract,
            )
            nc.vector.tensor_scalar(
                out=num2[:, :nbs], in0=cv[:, :nbs], scalar1=2.0, scalar2=C2,
                op0=OP.mult, op1=OP.add,
            )
            nc.vector.scalar_tensor_tensor(
                out=den2[:, :nbs], in0=var_i, scalar=C2, in1=var_j,
                op0=OP.add, op1=OP.add,
            )
            r2, S2, s12, sff = st("r2"), st("S2"), st("s12"), st("sff")
            nc.vector.reciprocal(out=r2[:, :nbs], in_=den2[:, :nbs])
            nc.vector.tensor_mul(out=S2[:, :nbs], in0=num2[:, :nbs], in1=r2[:, :nbs])
            nc.vector.tensor_mul(out=s12[:, :nbs], in0=S1[:, :nbs], in1=S2[:, :nbs])
            nc.vector.tensor_scalar(
                out=sff[:, :nbs], in0=s12[:, :nbs], scalar1=-1.0, scalar2=1.0,
                op0=OP.mult, op1=OP.add,
            )

            # fold sff into the matmul rhs: R2 = A*sff + xp_i + xp_j
            R = work.tile([128, K, 96], bf16, tag="R")
            nc.vector.tensor_mul(
                out=R[:, :nbs, :],
                in0=A[:, :nbs, :],
                in1=sff[:, :nbs].to_broadcast((128, nbs, 96)),
            )
            R2 = work.tile([128, K, 96], bf16, tag="R2")
            nc.vector.tensor_add(
                out=R2[:, :nbs, :], in0=R[:, :nbs, :], in1=U[:, :nbs, :]
            )

            po = psum.tile([128, 96], f32, tag="po")
            for b in range(nbs):
                nc.tensor.matmul(
                    out=po[:, :], lhsT=mb_sb[:, b, :], rhs=R2[:, b, :],
                    start=(b == 0), stop=(b == nbs - 1),
                )

            ot = outp.tile([128, 96], f32, tag="ot")
            nc.scalar.activation(out=ot[:nr, :], in_=po[:nr, :], func=AF.Copy)
            nc.sync.dma_start(out=out_d[n0 : n0 + nr, :], in_=ot[:nr, :])

    nc.compile()
    return nc


def _get_nc():
    global _nc_cache
    if _nc_cache is None:
        _nc_cache = _build_nc()
    return _nc_cache


def _build_idx(idx_i, idx_j):
    """idx_i/idx_j: [N, K] int -> [128, NCH, MAXCOLS] int16 wrapped layout.

    Per chunk the 2*nr*K indices (i-side then j-side) are split into 4
    contiguous block-ranges (one per SWDGE queue), each independently wrapped
    into 16 partitions and replicated across the 8 Q7 core pairs.
    """
    chunks = []
    for c in range(NCH):
        n0 = c * 128
        n1 = min(n0 + 128, N)
        comb = np.concatenate(
            [idx_i[n0:n1].reshape(-1), idx_j[n0:n1].reshape(-1)]
        ).astype(np.int16)
        nblocks = comb.size // 128
        cols = []
        off = 0
        for sz in _qsizes(nblocks):
            seg = comb[off * 128 : (off + sz) * 128]
            off += sz
            cols.append(seg.reshape(-1, 16).T)  # [16, sz*8]
        w = np.concatenate(cols, axis=1)
        full = np.tile(w, (8, 1))  # replicate across the 8 q7 cores
        if full.shape[1] < MAXCOLS:
            full = np.pad(full, ((0, 0), (0, MAXCOLS - full.shape[1])))
        chunks.append(full)
    return np.ascontiguousarray(np.stack(chunks, axis=1))


def _mbase():
    import ml_dtypes

    p = np.arange(128)[:, None, None]
    bb = np.arange(K)[None, :, None]
    m = np.arange(128)[None, None, :]
    mb = ((bb * 128 + p) // K == m).astype(np.float32)
    return np.ascontiguousarray(mb.astype(ml_dtypes.bfloat16))


def kernel(x, x_p, edge_index):
    from concourse.bass_utils import run_bass_kernel_spmd

    xs = np.ascontiguousarray(x[..., 0], dtype=np.float32)  # [B, C, N]
    xps = np.ascontiguousarray(x_p[..., 0], dtype=np.float32)
    idx_j_all = np.asarray(edge_index[0])  # neighbors
    idx_i_all = np.asarray(edge_index[1])  # centers
    mb = _mbase()

    in_maps = []
    for b in range(B):
        in_maps.append(
            {
                "x": xs[b],
                "xp": xps[b],
                "idx": _build_idx(idx_i_all[b], idx_j_all[b]),
                "mbase": mb,
            }
        )

    nc = _get_nc()
    res = run_bass_kernel_spmd(nc, in_maps, list(range(B))).results
    out = np.stack([r["out"].T for r in res])  # [B, C, N]
    return np.ascontiguousarray(out[..., None]).astype(np.float32)


if __name__ == "__main__":
    # quick smoke test with random data
    rng = np.random.default_rng(0)
    x = rng.standard_normal((B, C, N, 1), dtype=np.float32)
    x_p = rng.random((B, C, N, 1), dtype=np.float32)
    ei = rng.integers(0, N, size=(2, B, N, K)).astype(np.int32)
    out = kernel(x, x_p, ei)
    print(out.shape, out.dtype)
